# revision 1
# baseline (speedup 1.0000x reference)
"""Fake-attention kernel for trn2: 8 NeuronCores, one batch element per core.

Per core (batch b): out = softmax(k @ q^T) @ v, with k/q/v = x @ W.T + b.
The big matmuls (scores, PV) run as fp32r — full PE rate (1 col/cycle) at
free-dim >= 512 — giving ~1.6e-4 matmul noise; projections run in exact
fp32 (they are small) to keep end-to-end error ~5e-4.

Layout (everything transposed so softmax's reduction lands on the free axis
and the PV contraction lands on partitions, with no per-block transposes of
the probability matrix):
  xT [f,n]    <- PE-transpose of x chunks (exact, fp32)
  kT,qT [d,n] = W @ xT   (lhsT = W^T, pre-transposed on host, fp32 matmul)
  v [m,d]     = xT-chunks as lhsT, rhs = Wv^T  (natural layout)
  per n-section of 1024, streaming over m-chunks of 128:
    scoresT chunk [m=128, n=1024] = qT-slice as lhsT, kT as rhs (fp32r)
    pT = exp(scoresT)             (ACT, fp32r out, no max-subtraction:
                                   |scores| <= ~25 so fp32 exp is safe)
    outT [d,n] += v-chunk as lhsT, pT as rhs   (PSUM accumulation over m)
    denom[n] partial sums: two parallel chains (DVE evens / GPSIMD odds)
  finalize (deferred into the next section's stream):
    denom = per-block [d_even-slice]^T @ ones matmuls -> [n,1] columns
    out natural = PE-transpose(outT) * (1/denom) + bv

PSUM budget: 3x scores buffers [128,1024] (6 banks) + 1 PV accumulator
(2 banks). Emit order software-pipelines the PE one chunk ahead of ACT.
"""
import numpy as np

B = 8
N = 4096
D = 128
NC = 32          # chunks of 128 along n/m
NSEC = 4         # sections of 1024 along n
SEC = 1024

_cache = {}


def _build(defer_v=False, sec0_gp=False, early_merge=False,
           ptp_bufs=6, wrk_bufs=2, fin_b_at=6, gp_mod=2,
           spread_setup=True, last_merge=False, denom_mm=True,
           proj_f32=True, split_q0=True, hoist_x=True, tail_opt=True,
           defer_k=True, split_wp=True, x0_first=True, tp_up=True,
           fast_start=True, warmup_mms=4, fast_tail=False, merge_mm=True,
           last_mm=True):
    import concourse.bass as bass  # noqa
    import concourse.mybir as mybir
    import concourse.tile as tile
    from concourse import bacc

    F32 = mybir.dt.float32
    F32R = mybir.dt.float32r
    Exp = mybir.ActivationFunctionType.Exp
    AX = mybir.AxisListType.X
    ADD = mybir.AluOpType.add
    MUL = mybir.AluOpType.mult

    nc = bacc.Bacc()
    xt = nc.declare_dram_parameter("xt", [D, N], F32, isOutput=False)
    wp = nc.declare_dram_parameter("wp", [128, 643], F32, isOutput=False)
    y = nc.declare_dram_parameter("y", [N, D], F32, isOutput=True)

    xt_dram = xt.rearrange("p (c l) -> p c l", l=128)
    y_dram = y.rearrange("(c p) d -> p c d", p=128)

    with tile.TileContext(nc) as tc:
        with (
            tc.tile_pool(name="big", bufs=1) as big,
            tc.tile_pool(name="ptp", bufs=ptp_bufs) as ptp,
            tc.tile_pool(name="wrk", bufs=wrk_bufs) as wrk,
            tc.tile_pool(name="ps", bufs=3, space="PSUM") as psum,
            tc.tile_pool(name="ps1", bufs=1, space="PSUM") as psum1,
        ):
            xdt = F32 if proj_f32 else F32R
            if fast_start:
                # split the weight pack into separate tiles with DMAs ordered
                # by criticality: the first k-matmul only needs Wk^T + x0a
                xg0a = big.tile([128, 4, 128], xdt, tag="xT0a")
                xg0b = big.tile([128, 4, 128], xdt, tag="xT0b")
                wk_sb = big.tile([128, 128], F32, tag="wk")
                wq_sb = big.tile([128, 128], F32, tag="wq")
                wv_sb = big.tile([128, 129], F32, tag="wv")
                bvb_sb = big.tile([128, 128], F32, tag="bvb")
                bkq_sb = big.tile([128, 2], F32, tag="bkq")
                id_sb = big.tile([128, 128], F32, tag="id")
                nc.sync.dma_start(xg0a[:], xt_dram[:, 0:4, :])
                nc.sync.dma_start(wk_sb[:], wp[:, 128:256])
                nc.sync.dma_start(wq_sb[:], wp[:, 256:384])
                nc.sync.dma_start(bkq_sb[:], wp[:, 641:643])
                nc.sync.dma_start(xg0b[:], xt_dram[:, 4:8, :])
                nc.sync.dma_start(wv_sb[:], wp[:, 384:513])
                nc.sync.dma_start(bvb_sb[:], wp[:, 513:641])
                nc.sync.dma_start(id_sb[:], wp[:, 0:128])
                xg0 = (xg0a, xg0b)
                ident = id_sb[:]
                wkT = wk_sb[:]
                wqT = wq_sb[:]
                wvT = wv_sb[:, 0:128]
                ones_col = wv_sb[:, 128:129]
                bv_bc = bvb_sb[:]
                bk = bkq_sb[:, 0:1]
                bq = bkq_sb[:, 1:2]
            else:
                wp_sb = big.tile([128, 643], F32, tag="wp")
                xg0 = big.tile([128, 8, 128], xdt, tag="xT0")
                if x0_first:
                    nc.sync.dma_start(xg0[:], xt_dram[:, 0:8, :])
                if split_wp:
                    nc.sync.dma_start(wp_sb[:, 0:128], wp[:, 0:128])
                    nc.sync.dma_start(wp_sb[:, 128:643], wp[:, 128:643])
                else:
                    nc.sync.dma_start(wp_sb[:], wp[:])
                if not x0_first:
                    nc.sync.dma_start(xg0[:], xt_dram[:, 0:8, :])
                ident = wp_sb[:, 0:128]
                wkT = wp_sb[:, 128:256]
                wqT = wp_sb[:, 256:384]
                wvT = wp_sb[:, 384:512]
                ones_col = wp_sb[:, 512:513]
                bv_bc = wp_sb[:, 513:641]
                bk = wp_sb[:, 641:642]
                bq = wp_sb[:, 642:643]

            if warmup_mms:
                # warm the PE clock during the DMA wait: dummy fp32 matmuls
                # on a memset tile keep the array continuously busy so the
                # first real projection runs at full clock
                wu = big.tile([128, 128], F32, tag="warm")
                nc.vector.memset(wu[:], 1.0)
                wu_ps = psum.tile([128, 1024], F32, tag="sc")
                for _ in range(warmup_mms):
                    nc.tensor.matmul(wu_ps[:, 0:128], wu[:], wu[:],
                                     start=True, stop=True,
                                     skip_group_check=True)

            ones_r = big.tile([128, 1], F32R, tag="ones_r")
            nc.vector.tensor_copy(ones_r[:], ones_col)

            kT = [None] * 4
            qT = [None] * 4
            v_g = [None] * 4

            xT_g = [None] * 4

            def emit_setup_kq(g):
                emit_setup_x(g)
                xgf = xT_g[g].rearrange("p c f -> p (c f)")

                kg = big.tile([128, 1024], F32R, tag=f"kT{g}")
                psk = psum.tile([128, 1024], F32, tag="sc")
                nc.tensor.matmul(psk[:, 0:512], wkT, xgf[:, 0:512],
                                 start=True, stop=True)
                nc.tensor.matmul(psk[:, 512:1024], wkT, xgf[:, 512:1024],
                                 start=True, stop=True)
                nc.vector.tensor_scalar_add(kg[:], psk[:], bk)

                qg = big.tile([128, 1024], F32R, tag=f"qT{g}")
                psq = psum.tile([128, 1024], F32, tag="sc")
                nc.tensor.matmul(psq[:, 0:512], wqT, xgf[:, 0:512],
                                 start=True, stop=True)
                nc.tensor.matmul(psq[:, 512:1024], wqT, xgf[:, 512:1024],
                                 start=True, stop=True)
                nc.vector.tensor_scalar_add(qg[:], psq[:], bq)
                kT[g] = kg
                qT[g] = qg


            xT_g[0] = xg0

            def xslab(g, half):
                """[128, 512] slab of group g's xT (half = 0 or 1)."""
                xg = xT_g[g]
                if isinstance(xg, tuple):
                    return xg[half].rearrange("p c f -> p (c f)")
                return xg.rearrange("p c f -> p (c f)")[
                    :, half * 512:(half + 1) * 512]

            def xchunk(g, j):
                xg = xT_g[g]
                if isinstance(xg, tuple):
                    return xg[j // 4][:, j % 4, :]
                return xg[:, j, :]

            def emit_dma_x(g):
                xg = big.tile([128, 8, 128], F32 if proj_f32 else F32R,
                              tag=f"xT{g}")
                nc.sync.dma_start(xg[:], xt_dram[:, g * 8:(g + 1) * 8, :])
                xT_g[g] = xg

            def emit_tp_x(g):
                pass

            def emit_setup_x(g):
                if xT_g[g] is None:
                    emit_dma_x(g)

            def emit_setup_k(g):
                if fast_start and g == 0:
                    kga = big.tile([128, 512], F32R, tag="kT0a")
                    kgb = big.tile([128, 512], F32R, tag="kT0b")
                    pst = psum.tile([128, 1024], F32, tag="sc")
                    nc.tensor.matmul(pst[:, 0:512], wkT, xslab(g, 0),
                                     start=True, stop=True)
                    nc.vector.tensor_scalar_add(kga[:], pst[:, 0:512], bk)
                    nc.tensor.matmul(pst[:, 512:1024], wkT, xslab(g, 1),
                                     start=True, stop=True)
                    nc.vector.tensor_scalar_add(kgb[:], pst[:, 512:1024], bk)
                    kT[g] = (kga, kgb)
                    return
                tg = big.tile([128, 1024], F32R, tag=f"kT{g}")
                pst = psum.tile([128, 1024], F32, tag="sc")
                nc.tensor.matmul(pst[:, 0:512], wkT, xslab(g, 0),
                                 start=True, stop=True)
                nc.vector.tensor_scalar_add(tg[:, 0:512], pst[:, 0:512], bk)
                nc.tensor.matmul(pst[:, 512:1024], wkT, xslab(g, 1),
                                 start=True, stop=True)
                nc.vector.tensor_scalar_add(tg[:, 512:1024], pst[:, 512:1024], bk)
                kT[g] = tg

            def emit_setup_q(g):
                tg = big.tile([128, 1024], F32R, tag=f"qT{g}")
                pst = psum.tile([128, 1024], F32, tag="sc")
                nc.tensor.matmul(pst[:, 0:512], wqT, xslab(g, 0),
                                 start=True, stop=True)
                nc.vector.tensor_scalar_add(tg[:, 0:512], pst[:, 0:512], bq)
                nc.tensor.matmul(pst[:, 512:1024], wqT, xslab(g, 1),
                                 start=True, stop=True)
                nc.vector.tensor_scalar_add(tg[:, 512:1024], pst[:, 512:1024], bq)
                qT[g] = tg

            def emit_setup_v(g):
                vg = big.tile([128, 8, 128], F32R, tag=f"v{g}")
                psv = psum.tile([128, 1024], F32, tag="sc")
                for j in range(8):
                    nc.tensor.matmul(
                        psv[:, j * 128:(j + 1) * 128], xchunk(g, j), wvT,
                        start=True, stop=True,
                    )
                nc.vector.tensor_copy(vg[:], psv[:])
                v_g[g] = vg

            def emit_setup(g):
                emit_setup_kq(g)
                emit_setup_v(g)

            # denominator chain assignment; section 0 gives GPSIMD more (DVE
            # is busy with setup copies there). Chunks >= 28 stay on DVE so
            # the GP chain finishes early and the merge can be emitted before
            # the section's final chunks (shorter finalize tail).
            def chain_of(mc, sec):
                if mc >= 28 + (gp_mod - 2):
                    return "dve"
                if sec == 0 and sec0_gp:
                    return "gp" if (mc % 2 == 1 or mc % 8 == 2) else "dve"
                return "gp" if mc % gp_mod == 1 else "dve"

            def q_slice(mc):
                return qT[mc // 8][:, (mc % 8) * 128:(mc % 8 + 1) * 128]

            def v_chunk(mc):
                return v_g[mc // 8][:, mc % 8, :]

            if hoist_x:
                # x DMA for group 0 was the first DMA emitted (see below)
                for g in range(1, 4):
                    emit_dma_x(g)

            if split_q0:
                if hoist_x:
                    emit_tp_x(0)
                    if tp_up:
                        for g in range(1, 4):
                            emit_tp_x(g)
                else:
                    emit_setup_x(0)
                emit_setup_k(0)
                qg0 = big.tile([128, 1024], F32R, tag="qT0")
                psq0 = psum.tile([128, 1024], F32, tag="sc")
                nc.tensor.matmul(psq0[:, 0:128], wqT, xslab(0, 0)[:, 0:128],
                                 start=True, stop=True)
                nc.vector.tensor_scalar_add(qg0[:, 0:128], psq0[:, 0:128], bq)
                qT[0] = qg0
                pending_q0 = (qg0, psq0)
            elif defer_v:
                emit_setup_kq(0)
            else:
                emit_setup(0)

            pending_fin_a = [None]
            pending_fin_b = [None]

            def flush_fin_a():
                if pending_fin_a[0] is not None:
                    pending_fin_a[0]()
                    pending_fin_a[0] = None

            def flush_fin_b():
                if pending_fin_b[0] is not None:
                    pending_fin_b[0]()
                    pending_fin_b[0] = None

            pending_last = [None]

            def flush_pending_last():
                if pending_last[0] is not None:
                    pending_last[0]()
                    pending_last[0] = None

            for sec in range(NSEC):
                d_even = wrk.tile([128, 1024], F32, tag="de")
                d_odd = wrk.tile([128, 1024], F32, tag="do")

                def emit_scores(mc, sec=sec):
                    ps_s = psum.tile([128, 1024], F32, tag="sc")
                    q_sl = q_slice(mc)
                    kg = kT[sec]
                    if isinstance(kg, tuple):
                        ka, kb = kg
                    else:
                        ka, kb = kg[:, 0:512], kg[:, 512:1024]
                    nc.tensor.matmul(ps_s[:, 0:512], q_sl, ka,
                                     start=True, stop=True)
                    nc.tensor.matmul(ps_s[:, 512:1024], q_sl, kb,
                                     start=True, stop=True)
                    return ps_s

                def emit_exp(ps_s):
                    pT = ptp.tile([128, 1024], F32R, tag="pt")
                    nc.scalar.activation(pT[:], ps_s[:], Exp)
                    return pT

                # ---- first chunk of this section (emitted before the
                # previous section's last PV so the ACT never stalls at the
                # boundary)
                if sec == 0 and fast_start:
                    ka, kb = kT[0] if isinstance(kT[0], tuple) else (
                        kT[0][:, 0:512], kT[0][:, 512:1024])
                    q_sl = q_slice(0)
                    s0a = psum.tile([128, 1024], F32, tag="sc")
                    nc.tensor.matmul(s0a[:, 0:512], q_sl, ka,
                                     start=True, stop=True)
                    pTa = ptp.tile([128, 1024], F32R, tag="pt")
                    nc.scalar.activation(pTa[:, 0:512], s0a[:, 0:512], Exp)
                    s0b = psum.tile([128, 1024], F32, tag="sc")
                    nc.tensor.matmul(s0b[:, 0:512], q_sl, kb,
                                     start=True, stop=True)
                    pTb = ptp.tile([128, 1024], F32R, tag="pt")
                    nc.scalar.activation(pTb[:, 0:512], s0b[:, 0:512], Exp)
                    pT_prev = (pTa, pTb)
                else:
                    pT_prev = emit_exp(emit_scores(0))

                # close out the previous section, then claim its PV slot
                flush_pending_last()
                flush_fin_a()
                ps_pv = psum1.tile([128, 1024], F32, tag="pv")

                def emit_pv(mc, pT, ps_pv=ps_pv):
                    if isinstance(pT, tuple):
                        pa, pb = pT[0][:, 0:512], pT[1][:, 0:512]
                    else:
                        pa, pb = pT[:, 0:512], pT[:, 512:1024]
                    nc.tensor.matmul(
                        ps_pv[:, 0:512], v_chunk(mc), pa,
                        start=(mc == 0), stop=(mc == NC - 1),
                        skip_group_check=True,
                    )
                    nc.tensor.matmul(
                        ps_pv[:, 512:1024], v_chunk(mc), pb,
                        start=(mc == 0), stop=(mc == NC - 1),
                        skip_group_check=True,
                    )

                dve_chunks = [m for m in range(NC) if chain_of(m, sec) == "dve"]
                gp_chunks = [m for m in range(NC) if chain_of(m, sec) == "gp"]

                def emit_chain(mc, pT, d_even=d_even, d_odd=d_odd,
                               dve_chunks=dve_chunks, gp_chunks=gp_chunks,
                               sec=sec):
                    if last_mm and sec == NSEC - 1 and mc == NC - 1:
                        return  # folded into the denominator matmuls
                    if isinstance(pT, tuple):
                        pa = pT[0][:, 0:512].bitcast(F32)
                        pb = pT[1][:, 0:512].bitcast(F32)
                        assert chain_of(mc, sec) == "dve"
                        if mc == dve_chunks[0]:
                            nc.vector.tensor_copy(d_even[:, 0:512], pa)
                            nc.vector.tensor_copy(d_even[:, 512:1024], pb)
                        else:
                            nc.vector.tensor_tensor(
                                d_even[:, 0:512], d_even[:, 0:512], pa, ADD
                            )
                            nc.vector.tensor_tensor(
                                d_even[:, 512:1024], d_even[:, 512:1024], pb, ADD
                            )
                        return
                    pTf = pT.bitcast(F32)
                    if chain_of(mc, sec) == "dve":
                        if mc == dve_chunks[0]:
                            nc.vector.tensor_copy(d_even[:], pTf[:])
                        else:
                            nc.vector.tensor_tensor(
                                d_even[:], d_even[:], pTf[:], ADD
                            )
                    else:
                        if mc == gp_chunks[0]:
                            nc.gpsimd.tensor_copy(d_odd[:], pTf[:])
                        else:
                            nc.gpsimd.tensor_tensor(d_odd[:], d_odd[:], pTf[:], ADD)
                    # d_odd is complete after the last GP chunk; merge it into
                    # the DVE chain early so the section tail is shorter
                    if (early_merge or (last_merge and sec == NSEC - 1)) \
                            and mc == gp_chunks[-1] + 1:
                        nc.vector.tensor_tensor(d_even[:], d_even[:], d_odd[:], ADD)

                if sec == 0 and split_q0:
                    qg0, psq0 = pending_q0
                    nc.tensor.matmul(psq0[:, 128:512], wqT,
                                     xslab(0, 0)[:, 128:512],
                                     start=True, stop=True)
                    nc.tensor.matmul(psq0[:, 512:1024], wqT, xslab(0, 1),
                                     start=True, stop=True)
                    nc.vector.tensor_scalar_add(
                        qg0[:, 128:1024], psq0[:, 128:1024], bq
                    )
                    emit_setup_v(0)
                for mc in range(1, NC):
                    # interleave remaining setup groups into section 0;
                    # v-projections are deferred until just before their
                    # first PV use so the first exp starts sooner
                    if sec == 0 and spread_setup:
                        g = mc // 8 + 1
                        if g < 4:
                            r = mc % 8
                            if r == 1:
                                if hoist_x and not tp_up:
                                    emit_tp_x(g)
                                elif not hoist_x:
                                    emit_setup_x(g)
                            elif r == 3 and (g < 2 or not defer_k):
                                emit_setup_k(g)
                            elif r == 5:
                                emit_setup_q(g)
                            elif r == 7:
                                emit_setup_v(g)
                    elif defer_k and sec in (1, 2) and mc == 16:
                        emit_setup_k(sec + 1)
                    elif sec == 0 and mc % 8 == 1:
                        if defer_v:
                            emit_setup_v(mc // 8)
                            if mc // 8 + 1 < 4:
                                emit_setup_kq(mc // 8 + 1)
                        else:
                            if mc // 8 + 1 < 4:
                                emit_setup(mc // 8 + 1)
                    if fast_tail and sec == NSEC - 1 and mc == NC - 1:
                        # last chunk of the last section: half-width pipeline
                        # so the denominator add overlaps the second exp
                        kg = kT[sec]
                        ka, kb = (kg if isinstance(kg, tuple)
                                  else (kg[:, 0:512], kg[:, 512:1024]))
                        q_sl = q_slice(mc)
                        s_a = psum.tile([128, 1024], F32, tag="sc")
                        nc.tensor.matmul(s_a[:, 0:512], q_sl, ka,
                                         start=True, stop=True)
                        emit_pv(mc - 1, pT_prev)
                        emit_chain(mc - 1, pT_prev)
                        pTa = ptp.tile([128, 1024], F32R, tag="pt")
                        nc.scalar.activation(pTa[:, 0:512], s_a[:, 0:512], Exp)
                        s_b = psum.tile([128, 1024], F32, tag="sc")
                        nc.tensor.matmul(s_b[:, 0:512], q_sl, kb,
                                         start=True, stop=True)
                        pTb = ptp.tile([128, 1024], F32R, tag="pt")
                        nc.scalar.activation(pTb[:, 0:512], s_b[:, 0:512], Exp)
                        pT_prev = (pTa, pTb)
                        continue
                    ps_s = emit_scores(mc)
                    emit_pv(mc - 1, pT_prev)
                    emit_chain(mc - 1, pT_prev)
                    if mc == fin_b_at:
                        flush_fin_b()
                    pT_prev = emit_exp(ps_s)
                def make_last(pv=emit_pv, ch=emit_chain, p=pT_prev):
                    def last():
                        pv(NC - 1, p)
                        ch(NC - 1, p)
                    return last

                pending_last[0] = make_last()

                fin_state = {}

                def make_fin_a(sec=sec, ps_pv=ps_pv, fin_state=fin_state,
                               d_even=d_even, d_odd=d_odd):
                    def fin_a():
                        if not (merge_mm or early_merge
                                or (last_merge and sec == NSEC - 1)):
                            nc.vector.tensor_tensor(
                                d_even[:], d_even[:], d_odd[:], ADD
                            )
                        o_copy = wrk.tile([128, 1024], F32, tag="oc")
                        if tail_opt and sec == NSEC - 1:
                            nc.scalar.copy(o_copy[:], ps_pv[:])
                        else:
                            nc.vector.tensor_copy(o_copy[:], ps_pv[:])
                        fin_state["o_copy"] = o_copy
                    return fin_a

                def make_fin_b(sec=sec, d_even=d_even, d_odd=d_odd,
                               fin_state=fin_state, pT31=pT_prev):
                    def fin_b():
                        o_copy = fin_state["o_copy"]
                        recip = wrk.tile([128, 8], F32, tag="rc")
                        if denom_mm:
                            ones_f32 = ones_col
                            tpd = psum.tile([128, 1024], F32, tag="sc")
                            for nb in range(8):
                                sl = slice(nb * 128, (nb + 1) * 128)
                                nc.tensor.matmul(
                                    tpd[:, nb:nb + 1], d_even[:, sl], ones_f32,
                                    start=True, stop=not merge_mm,
                                    skip_group_check=True,
                                )
                                lastm = (last_mm and sec == NSEC - 1)
                                if merge_mm:
                                    nc.tensor.matmul(
                                        tpd[:, nb:nb + 1], d_odd[:, sl],
                                        ones_f32, start=False, stop=not lastm,
                                        skip_group_check=True,
                                    )
                                if lastm:
                                    p31 = (pT31[nb // 4][:, sl.start % 512:
                                                         sl.start % 512 + 128]
                                           if isinstance(pT31, tuple)
                                           else pT31[:, sl])
                                    nc.tensor.matmul(
                                        tpd[:, nb:nb + 1], p31.bitcast(F32),
                                        ones_f32, start=False, stop=True,
                                        skip_group_check=True,
                                    )
                            nc.vector.reciprocal(recip[:], tpd[:, 0:8])
                        else:
                            tpd = psum.tile([128, 1024], F32, tag="sc")
                            for nb in range(8):
                                sl = slice(nb * 128, (nb + 1) * 128)
                                nc.tensor.transpose(tpd[:, sl], d_even[:, sl], ident)
                            denom = wrk.tile([128, 8], F32, tag="dn")
                            nc.vector.reduce_sum(
                                denom[:], tpd.rearrange("p (b l) -> p b l", b=8),
                                axis=AX,
                            )
                            nc.vector.reciprocal(recip[:], denom[:])

                        out_g = big.tile([128, 8, 128], F32, tag=f"out{sec}")
                        bv_bcx4 = bv_bc[:, None, :].to_broadcast((128, 4, 128))
                        halves = 2 if (tail_opt and sec == NSEC - 1) else 1
                        for h in range(halves):
                            lo, hi = h * 8 // halves, (h + 1) * 8 // halves
                            nblk = hi - lo
                            tpo = psum.tile([128, 1024], F32, tag="sc")
                            for nb in range(lo, hi):
                                sl = slice(nb * 128, (nb + 1) * 128)
                                nc.tensor.transpose(
                                    tpo[:, (nb - lo) * 128:(nb - lo + 1) * 128],
                                    o_copy[:, sl], ident,
                                )
                            tpo_v = tpo[:, 0:nblk * 128].rearrange(
                                "p (b l) -> p b l", b=nblk
                            )
                            o_sl = out_g[:, lo:hi, :]
                            recip_bc = recip[:, lo:hi, None].to_broadcast(
                                (128, nblk, 128)
                            )
                            bv_bcx = bv_bc[:, None, :].to_broadcast(
                                (128, nblk, 128)
                            )
                            nc.vector.tensor_tensor(o_sl, tpo_v, recip_bc, MUL)
                            nc.vector.tensor_tensor(o_sl, o_sl, bv_bcx, ADD)
                            nc.sync.dma_start(
                                y_dram[:, sec * 8 + lo:sec * 8 + hi, :], o_sl
                            )
                    return fin_b

                pending_fin_a[0] = make_fin_a()
                pending_fin_b[0] = make_fin_b()

            flush_pending_last()
            flush_fin_a()
            flush_fin_b()

    nc.finalize()
    return nc


def _get_nc():
    if "nc" not in _cache:
        _cache["nc"] = _build()
    return _cache["nc"]


def make_wp(Wk, Wq, Wv, bk, bq, bv):
    wp = np.zeros((128, 643), np.float32)
    wp[:, 0:128] = np.eye(128, dtype=np.float32)
    wp[:, 128:256] = Wk.T
    wp[:, 256:384] = Wq.T
    wp[:, 384:512] = Wv.T
    wp[:, 512] = 1.0
    wp[:, 513:641] = np.broadcast_to(bv[None, :], (128, 128))
    wp[:, 641] = bk
    wp[:, 642] = bq
    return wp


def kernel(x, Wk, bk, Wq, bq, Wv, bv, **_ignored):
    from concourse.bass_utils import run_bass_kernel_spmd

    x = np.asarray(x, dtype=np.float32)
    wp = make_wp(
        np.asarray(Wk, np.float32), np.asarray(Wq, np.float32),
        np.asarray(Wv, np.float32), np.asarray(bk, np.float32),
        np.asarray(bq, np.float32), np.asarray(bv, np.float32),
    )

    nc = _get_nc()
    in_maps = [
        {"xt": np.ascontiguousarray(x[b].T), "wp": wp} for b in range(B)
    ]
    res = run_bass_kernel_spmd(nc, in_maps, core_ids=list(range(B)))
    out = np.stack([res.results[b]["y"] for b in range(B)], axis=0)
    return out



# revision 28
# speedup vs baseline: 1.1877x; 1.1877x over previous
"""Fake-attention kernel for trn2: 8 NeuronCores, one batch element per core.

Per core (batch b): out = softmax(k @ q^T) @ v with k/q/v = x @ W.T + b.

Key algebraic reduction: softmax rows are shift-invariant, and
  k_n . q_m = (Wq^T k_n) . x_m + (k_n . bq)
where the second term is constant along the softmax axis m.  So with
  kpp = (Wq^T Wk) x + Wq^T bk      (ONE projection instead of k and q)
softmax(k q^T) == softmax(kpp x^T) exactly.  The q-side of the scores
matmul is the raw (already-resident) xT.

Layout: transposed so the softmax reduction lands on PE partitions and
PV contracts on partitions:
  xT   [f, n]   (host-transposed input)
  kppT [d, n] = Ck^T @ xT + g      (fp32r, exact)
  v    [m, d] = x @ Wv^T           (bf16 inputs, fp32 psum, stored f32r)
  scoresT chunk [m=128, n=512] = xT-chunk as lhsT, kppT-slice as rhs (fp32r)
  p = exp(scoresT), ONE 1536-wide ACT op per 3 chunks (bf16 out)
  outT [d, n-sec] += v-chunk as lhsT, p-chunk as rhs  (PSUM accumulation)

ACT (the only exp engine) is the bottleneck: 131072 exp columns at
0.833ns/col + ~185ns/instruction.  Everything else is arranged to keep
ACT saturated: 86 exp ops of width 1536; PSUM = 2x[128,1536] score
tiles (double buffer) + 2x[128,512] persistent PV accumulators (even/odd
sections); denominators off ACT (DVE bf16 pair-trees at 2x + Pool fp32
chains); v/kpp setup matmuls slotted into PV-accumulator idle windows;
finalize transposes reuse the retired PV psum tile.
"""
import numpy as np

B = 8
N = 4096
D = 128
SEC = 512            # n-section width (PV accumulator width)
NSEC = N // SEC      # 8
NCH = 32             # m-chunks of 128 per section
NG = NSEC * NCH      # 256 chunk units
TCH = 3              # chunks per bulk exp tile
NT = 88              # tiles: 1 + 84*3 + 3*1 = 256 chunks

_cache = {}

# wp pack layout (columns)
_CK = slice(0, 128)        # Ck = Wk^T Wq  (lhsT for kpp projection)
_G = slice(128, 129)       # g = Wq^T bk
_ONES = slice(129, 130)    # 1.0 column
_WVT = slice(130, 258)     # Wv^T
_BVB = slice(258, 386)     # bv broadcast [128, 128]
_ID = slice(386, 514)      # identity (PE transpose)
WPW = 514


def make_wp(Wk, Wq, Wv, bk, bq, bv):
    wp = np.zeros((128, WPW), np.float32)
    wp[:, _CK] = Wk.T @ Wq
    wp[:, _G] = (Wq.T @ bk)[:, None]
    wp[:, _ONES] = 1.0
    wp[:, _WVT] = Wv.T
    wp[:, _BVB] = np.broadcast_to(bv[None, :], (128, 128))
    wp[:, _ID] = np.eye(128, dtype=np.float32)
    return wp


def _build(pool_mod=4, ptp_bufs=6, tree_bufs=14,
           fin_a_at=3, fin_b_at=5):
    import concourse.bass as bass  # noqa
    import concourse.mybir as mybir
    import concourse.tile as tile
    from concourse import bacc

    F32 = mybir.dt.float32
    F32R = mybir.dt.float32r
    BF16 = mybir.dt.bfloat16
    Exp = mybir.ActivationFunctionType.Exp
    ADD = mybir.AluOpType.add
    MUL = mybir.AluOpType.mult

    nc = bacc.Bacc()
    xt = nc.declare_dram_parameter("xt", [D, N], F32R, isOutput=False)
    wp = nc.declare_dram_parameter("wp", [128, WPW], F32R, isOutput=False)
    y = nc.declare_dram_parameter("y", [N, D], F32, isOutput=True)

    xt_dram = xt.rearrange("p (c l) -> p c l", l=128)
    y_dram = y.rearrange("(c p) d -> p c d", p=128)

    with tile.TileContext(nc) as tc:
        with (
            tc.tile_pool(name="big", bufs=1) as big,
            tc.tile_pool(name="ptp", bufs=ptp_bufs) as ptp,
            tc.tile_pool(name="tree", bufs=tree_bufs) as tree,
            tc.tile_pool(name="dop", bufs=5) as dop,
            tc.tile_pool(name="wrk", bufs=4) as wrk,
            tc.tile_pool(name="ts", bufs=2, space="PSUM") as tpool,
            tc.tile_pool(name="pva", bufs=1, space="PSUM") as pvpa,
            tc.tile_pool(name="pvb", bufs=1, space="PSUM") as pvpb,
        ):
            # ---------------- startup DMAs (criticality order) ----------
            wp_sb = big.tile([128, WPW], F32R, tag="wp")
            xg = [None] * 4       # xT group tiles [128, 8, 128] f32
            xg0a = big.tile([128, 4, 128], F32R, tag="xT0a")
            xg0b = big.tile([128, 4, 128], F32R, tag="xT0b")
            nc.sync.dma_start(xg0a[:, 0:2, :], xt_dram[:, 0:2, :])
            nc.scalar.dma_start(wp_sb[:, 0:130], wp[:, 0:130])
            nc.sync.dma_start(xg0a[:, 2:4, :], xt_dram[:, 2:4, :])
            nc.sync.dma_start(xg0b[:], xt_dram[:, 4:8, :])
            nc.scalar.dma_start(wp_sb[:, 130:WPW], wp[:, 130:WPW])
            for gi in range(1, 4):
                t = big.tile([128, 8, 128], F32R, tag=f"xT{gi}",
                             name=f"xT{gi}")
                nc.sync.dma_start(t[:], xt_dram[:, gi * 8:(gi + 1) * 8, :])
                xg[gi] = t

            ckT = wp_sb[:, _CK]
            gcol = wp_sb[:, _G].bitcast(F32)
            ident = wp_sb[:, _ID]
            bv_bc = wp_sb[:, _BVB].bitcast(F32)

            def xchunk(mc):
                """[128, 128] f32r slab of xT for m-chunk mc."""
                if mc < 4:
                    return xg0a[:, mc, :]
                if mc < 8:
                    return xg0b[:, mc - 4, :]
                return xg[mc // 8][:, mc % 8, :]

            def xslab(gi, half):
                """[128, 512] f32r slab (half of group gi)."""
                if gi == 0:
                    t = xg0a if half == 0 else xg0b
                    return t.rearrange("p c f -> p (c f)")
                return xg[gi].rearrange("p c f -> p (c f)")[
                    :, half * 512:(half + 1) * 512]

            # ---------------- PE warmup (clock ramp) --------------------
            wu = big.tile([128, 128], F32, tag="warm")
            nc.vector.memset(wu[:], 1.0)
            tsA = tpool.tile([128, 1536], F32, tag="ts")

            def warm(n=1):
                for _ in range(n):
                    nc.tensor.matmul(tsA[:, 0:128], wu[:], wu[:],
                                     start=True, stop=True,
                                     skip_group_check=True)
            warm(2)

            # bf16 helper tensors
            wv_bf = big.tile([128, 128], BF16, tag="wvbf")
            ones_bf = big.tile([128, 1], BF16, tag="onesbf")

            # kppT [128, 8, 512] f32r; per-section slices
            kpp = big.tile([128, 8, 512], F32R, tag="kpp")
            # v groups [128, 8, 128] bf16 (PV pairs with bf16 p)
            v_g = [big.tile([128, 8, 128], BF16, tag=f"v{gi}",
                            name=f"v{gi}") for gi in range(4)]
            # bf16 copies of x groups (v projection lhsT)
            xbf = [big.tile([128, 8, 128], BF16, tag=f"xbf{gi}",
                            name=f"xbf{gi}") for gi in range(4)]

            def v_chunk(mc):
                return v_g[mc // 8][:, mc % 8, :]

            def emit_kpp(s, ps, off):
                """kpp projection for section s into ps[:, off:off+512]."""
                nc.tensor.matmul(ps[:, off:off + 512], ckT,
                                 xslab(s // 2, s % 2), start=True, stop=True)
                nc.vector.tensor_scalar_add(kpp[:, s, :], ps[:, off:off + 512],
                                            gcol)

            def emit_vhalf(h, ps, off, copy_eng):
                """v chunks 4h..4h+3 into ps[:, off:off+512], copy to v_g."""
                for j in range(4):
                    mc = 4 * h + j
                    nc.tensor.matmul(
                        ps[:, off + j * 128:off + (j + 1) * 128],
                        xbf[mc // 8][:, mc % 8, :], wv_bf[:],
                        start=True, stop=True)
                dst = v_g[h // 2][:, (h % 2) * 4:(h % 2) * 4 + 4, :]
                src = ps[:, off:off + 512].rearrange("p (c f) -> p c f", f=128)
                nc.vector.tensor_copy(dst, src)

            # ---------------- startup compute ---------------------------
            nc.gpsimd.tensor_copy(wv_bf[:], wp_sb[:, _WVT])
            nc.gpsimd.tensor_copy(ones_bf[:], wp_sb[:, _ONES])
            nc.gpsimd.tensor_copy(xbf[0][:, 0:4, :], xg0a[:])
            warm(2)
            # startup tile A: warm region | kpp sec0 (split halves so the
            # first scores chunk starts as early as possible)
            nc.tensor.matmul(tsA[:, 512:768], ckT, xslab(0, 0)[:, 0:256],
                             start=True, stop=True)
            nc.vector.tensor_scalar_add(kpp[:, 0, 0:256], tsA[:, 512:768],
                                        gcol)
            nc.tensor.matmul(tsA[:, 1024:1280], ckT,
                             xslab(0, 0)[:, 256:512],
                             start=True, stop=True)
            nc.vector.tensor_scalar_add(kpp[:, 0, 256:512],
                                        tsA[:, 1024:1280], gcol)
            nc.gpsimd.tensor_copy(xbf[0][:, 4:8, :], xg0b[:])

            # persistent PV accumulators: even sections -> accA, odd -> accB
            accA = pvpa.tile([128, 512], F32, tag="pvA")
            accB = pvpb.tile([128, 512], F32, tag="pvB")

            def pv_acc(s):
                return accA if s % 2 == 0 else accB

            # deferred work queue: (due_tile, fn), flushed in due order
            pending = []

            def flush_due(t):
                i = 0
                while i < len(pending):
                    due, fn = pending[i]
                    if due <= t:
                        fn()
                        pending.pop(i)
                    else:
                        i += 1

            # denominator state per live section
            dstate = {}

            def new_dstate(s):
                dstate[s] = {"partials": [], "odd": None}

            new_dstate(0)

            def emit_chain(g, p_slice):
                """Route chunk g's denominator contribution.  The last
                section routes Pool chunks early (mc<16) and pre-collapses
                the tree at mc==29 so the post-exp tail is short."""
                s = g // NCH
                mc = g % NCH
                st = dstate[s]
                last = (s == NSEC - 1)
                is_pool = ((mc % 2 == 1 and mc < 16) if last
                           else mc % pool_mod == pool_mod - 1)
                if is_pool:
                    if st["odd"] is None:
                        st["odd"] = dop.tile([128, 512], F32, tag="dodd", name="dodd")
                        nc.gpsimd.tensor_copy(st["odd"][:], p_slice)
                    else:
                        nc.gpsimd.tensor_tensor(st["odd"][:], st["odd"][:],
                                                p_slice, ADD)
                    return
                if last:
                    # two independent sequential bf16 chains: each add is
                    # gated only by its p slice and the chain's previous add
                    # (~a tile apart), so nothing piles up at the end.  The
                    # final chunks land on cB; the only post-final-exp ops
                    # are cB's last add and the cA+cB fold in fin_a.
                    key = "cB" if (mc % 2 == 1 or mc >= 30) else "cA"
                    chain = st.get(key)
                    if chain is None:
                        t2 = dop.tile([128, 512], BF16, tag=key, name=key)
                        nc.vector.tensor_copy(t2[:], p_slice)
                        st[key] = t2
                    else:
                        nc.vector.tensor_tensor(chain[:], chain[:],
                                                p_slice, ADD)
                    if mc == 17:
                        # Pool's fp32 chain (mc<16) is complete; fold into cA
                        nc.vector.tensor_tensor(st["cA"][:], st["cA"][:],
                                                st["odd"][:], ADD)
                        st["odd"] = None
                    return
                # DVE bf16 binary-counter tree (2x mode: all-bf16 SBUF)
                parts = st["partials"]
                parts.append((p_slice, 0))
                while len(parts) >= 2 and parts[-1][1] == parts[-2][1]:
                    a, lv = parts.pop()
                    b, _ = parts.pop()
                    t2 = tree.tile([128, 512], BF16, tag="dt")
                    nc.vector.tensor_tensor(t2[:], a, b, ADD)
                    parts.append((t2[:], lv + 1))

            def emit_fin_a(s):
                """Close section s: merge denominators, partition-sum matmuls
                into the retired PV tile, reciprocal, o_copy."""
                st = dstate.pop(s)
                acc = pv_acc(s)
                if "cA" in st:
                    dfin_t = tree.tile([128, 512], BF16, tag="dfin")
                    nc.vector.tensor_tensor(dfin_t[:], st["cA"][:],
                                            st["cB"][:], ADD)
                    st["partials"] = [(dfin_t[:], 0)]
                    st["odd"] = None
                parts = st["partials"]
                assert parts, f"empty denominator state for section {s}"
                while len(parts) > 1:
                    a, _ = parts.pop()
                    b, lv = parts.pop()
                    t2 = tree.tile([128, 512], BF16, tag="dt")
                    nc.vector.tensor_tensor(t2[:], a, b, ADD)
                    parts.append((t2[:], lv + 1))
                if st["odd"] is not None:
                    dfin = tree.tile([128, 512], BF16, tag="dfin")
                    nc.vector.tensor_tensor(dfin[:], parts[0][0],
                                            st["odd"][:], ADD)
                    dfin = dfin[:]
                else:
                    dfin = parts[0][0]
                o_copy = wrk.tile([128, 512], F32R, tag="oc")
                nc.vector.tensor_copy(o_copy[:], acc[:])
                # last section: tpd into an idle T-pool tile (survives the
                # transposes, which reuse acc) and divide directly -- no
                # reciprocal on the tail critical path
                if s == NSEC - 1:
                    tpd = tpool.tile([128, 1536], F32, tag="ts")
                else:
                    tpd = acc
                for b_ in range(4):
                    nc.tensor.matmul(
                        tpd[:, b_:b_ + 1],
                        dfin[:, b_ * 128:(b_ + 1) * 128], ones_bf[:],
                        start=True, stop=True, skip_group_check=True)
                recip = wrk.tile([128, 4], F32, tag="rc")
                nc.vector.reciprocal(recip[:], tpd[:, 0:4])
                return {"o_copy": o_copy, "recip": recip, "s": s}

            def emit_fin_b(fs):
                """Transposes + scale + bias + DMA out for section fs['s']."""
                s = fs["s"]
                acc = pv_acc(s)
                o_copy = fs["o_copy"]
                for b_ in range(4):
                    nc.tensor.transpose(
                        acc[:, b_ * 128:(b_ + 1) * 128].bitcast(F32R),
                        o_copy[:, b_ * 128:(b_ + 1) * 128],
                        ident)
                out_g = wrk.tile([128, 4, 128], F32, tag="og")
                tpo_v = acc[:, 0:512].rearrange("p (b l) -> p b l", b=4)
                recip = fs["recip"]
                for b_ in range(4):
                    nc.vector.scalar_tensor_tensor(
                        out_g[:, b_, :], tpo_v[:, b_, :], recip[:, b_:b_ + 1],
                        bv_bc, MUL, ADD)
                    if b_ == 1:
                        nc.sync.dma_start(
                            y_dram[:, s * 4:s * 4 + 2, :], out_g[:, 0:2, :])
                nc.sync.dma_start(
                    y_dram[:, s * 4 + 2:s * 4 + 4, :], out_g[:, 2:4, :])

            def emit_pv_and_chain(p_tile, chunks):
                for j, g in enumerate(chunks):
                    s = g // NCH
                    mc = g % NCH
                    if mc == 0 and s not in dstate:
                        new_dstate(s)
                    psl = p_tile[:, j * 512:(j + 1) * 512]
                    nc.tensor.matmul(
                        pv_acc(s)[:], v_chunk(mc), psl,
                        start=(mc == 0), stop=(mc == NCH - 1),
                        skip_group_check=True)
                    emit_chain(g, psl)
                    if mc == NCH - 1:
                        fs_box = {}

                        def fa(fs_box=fs_box, sv=s):
                            fs_box["fs"] = emit_fin_a(sv)

                        def fb(fs_box=fs_box):
                            emit_fin_b(fs_box["fs"])
                        tcur = tile_of(g)
                        pending.append((tcur + fin_a_at, fa))
                        pending.append((tcur + fin_b_at, fb))

            # staged setup: kpp1 + v halves 0..7 ride accB before its first
            # PV use (section 1 starts ~tile 11); kpp 2..7 ride accA idle
            # windows right after each even section's finalize.
            def setup_step(step):
                def run():
                    if step == -1:
                        emit_kpp(1, accB, 0)
                    elif 0 <= step < 8:
                        emit_vhalf(step, accB, 0, "gp")
                    elif step == 8:
                        emit_kpp(2, accA, 0)
                        emit_kpp(3, accA, 0)
                    elif step == 9:
                        emit_kpp(4, accA, 0)
                        emit_kpp(5, accA, 0)
                    elif step == 10:
                        emit_kpp(6, accA, 0)
                        emit_kpp(7, accA, 0)
                return run

            pending.append((0, setup_step(-1)))
            for step in range(8):
                pending.append((step, setup_step(step)))

            def xbf_copy(gi):
                def run():
                    nc.vector.tensor_copy(xbf[gi][:], xg[gi][:])
                return run
            pending.append((0, xbf_copy(1)))
            pending.append((2, xbf_copy(2)))
            pending.append((5, xbf_copy(3)))
            # accA windows: after fin_b(0) ~tile 10+fin_b_at, after
            # fin_b(2) ~tile 31+fin_b_at, after fin_b(4) ~tile 53+fin_b_at.
            pending.append((11 + fin_b_at + 1, setup_step(8)))
            pending.append((33 + fin_b_at + 1, setup_step(9)))
            pending.append((54 + fin_b_at + 1, setup_step(10)))

            # ---------------- main stream -------------------------------
            # Emission order per iteration t:
            #   exp(t) [ACT] ; scores(t+1) [PE, gated on exp(t-1) via the
            #   T-buffer rotation -- runs immediately when exp(t-1) ends] ;
            #   deferred setup/finalize ; PV+chain(t-1) [gated on exp(t-1)].
            # This keeps scores(t+1) AHEAD of PV(t-1) on the in-order PE
            # stream so exp(t+1) is never starved.
            def tile_chunks(t):
                # tile 0: single chunk (fast first exp); tiles 1..84: three
                # chunks; tiles 85..87: single chunks (short post-exp tail).
                if t == 0:
                    return [0]
                if t <= 84:
                    return list(range(3 * t - 2, 3 * t + 1))
                return [252 + (t - 84)]

            def tile_of(g):
                if g == 0:
                    return 0
                if g <= 252:
                    return (g + 2) // 3
                return 84 + (g - 252)

            def emit_scores(t):
                chunks = tile_chunks(t)
                ts = tpool.tile([128, 1536], F32, tag="ts")
                for j, g in enumerate(chunks):
                    if t == 0:
                        nc.tensor.matmul(ts[:, 0:256], xchunk(0),
                                         kpp[:, 0, 0:256],
                                         start=True, stop=True)
                        nc.tensor.matmul(ts[:, 256:512], xchunk(0),
                                         kpp[:, 0, 256:512],
                                         start=True, stop=True)
                    else:
                        nc.tensor.matmul(ts[:, j * 512:(j + 1) * 512],
                                         xchunk(g % NCH), kpp[:, g // NCH, :],
                                         start=True, stop=True)
                return ts, chunks

            cur = emit_scores(0)
            lag = []        # (p_tile, chunks) awaiting PV + chain, depth 2

            for t in range(NT):
                ts, chunks = cur
                p = ptp.tile([128, 1536], BF16, tag="pt")
                w = len(chunks) * 512
                nc.scalar.activation(p[:, 0:w], ts[:, 0:w], Exp)
                if t + 1 < NT:
                    cur = emit_scores(t + 1)
                flush_due(t)
                lag.append((p, chunks))
                if len(lag) > 2:
                    emit_pv_and_chain(*lag.pop(0))

            # drain
            for item in lag:
                emit_pv_and_chain(*item)
                flush_due(NT + 10)
            flush_due(NT + 1000)

    nc.finalize()
    return nc


def _get_nc():
    if "nc" not in _cache:
        _cache["nc"] = _build()
    return _cache["nc"]


def kernel(x, Wk, bk, Wq, bq, Wv, bv, **_ignored):
    from concourse.bass_utils import run_bass_kernel_spmd

    x = np.asarray(x, dtype=np.float32)
    wp = make_wp(
        np.asarray(Wk, np.float32), np.asarray(Wq, np.float32),
        np.asarray(Wv, np.float32), np.asarray(bk, np.float32),
        np.asarray(bq, np.float32), np.asarray(bv, np.float32),
    )

    nc = _get_nc()
    in_maps = [
        {"xt": np.ascontiguousarray(x[b].T), "wp": wp} for b in range(B)
    ]
    res = run_bass_kernel_spmd(nc, in_maps, core_ids=list(range(B)))
    out = np.stack([res.results[b]["y"] for b in range(B)], axis=0)
    return out
